# revision 14
# baseline (speedup 1.0000x reference)
"""Luong concat attention kernel for Trainium2, data-parallel over batch on 8 cores.

Reference computation (per batch row b):
    proj   = enc[b] @ W_e + dec[b] @ W_d        # [S, A]
    energy = tanh(proj)                          # [S, A]
    scores = energy @ v                          # [S]        (output 1)
    probs  = softmax(scores)
    ctx    = probs @ enc[b]                      # [D]        (output 2)

Device dataflow (per core, 4 batches of S=4096, D=A=512):
  - proj computed transposed (projT[a, s]) on PE: lhsT = W_e chunk [d,a]
    (stationary), rhs = encT chunk [d, s] (bf16, pre-transposed on host).
  - dec[b] @ W_d is 0.025% of the FLOPs -> computed on host, folded into the
    tanh as the ACT engine's per-partition bias (projT has a on partitions).
  - scores = v . energyT : M=1 matvecs on PE, col-tiled to partitions
    {0,32,64,96} of two PSUM banks (4 concurrent col-groups).
  - exp on ACT with fused accum_out row sums (softmax denominator partials).
  - exps transposed to [s-partition, chunk] layout via 4 PE transposes, then
    context = exps . enc (natural bf16 layout) as M=1 matvecs, col-tiled into
    4 partial sums.
  - Host epilogue: ctx = partials.sum(0) / Z  (unshard step).
"""

import numpy as np
import ml_dtypes

import concourse.bass as bass
import concourse.bacc as bacc
import concourse.tile as tile
import concourse.mybir as mybir

BF16 = mybir.dt.bfloat16
F32 = mybir.dt.float32
AF = mybir.ActivationFunctionType

N_CORES = 8
B, S, D, A = 32, 4096, 512, 512
BPC = B // N_CORES          # batches per core = 4
P = 128
NDC = D // P                # 4 contraction chunks
NAC = A // P                # 4 a chunks
NST = S // 512              # 8 s tiles of 512
NSC = S // P                # 32 s chunks of 128


def build_nc():
    nc = bacc.Bacc("TRN2", target_bir_lowering=False, debug=False,
                   num_devices=N_CORES)

    # inputs (per core)
    encT_d = nc.dram_tensor("encT", [BPC, D, S], BF16, kind="ExternalInput")
    encN_d = nc.dram_tensor("encN", [BPC, S, D], BF16, kind="ExternalInput")
    we_d = nc.dram_tensor("we", [D, A], BF16, kind="ExternalInput")
    v_d = nc.dram_tensor("vv", [P, NAC], BF16, kind="ExternalInput")
    bias_d = nc.dram_tensor("biasT", [P, NAC * BPC], F32, kind="ExternalInput")
    id_d = nc.dram_tensor("ident", [P, P], BF16, kind="ExternalInput")

    # outputs (per core)
    scores_o = nc.dram_tensor("scores_o", [BPC, S], F32, kind="ExternalOutput")
    ctxp_o = nc.dram_tensor("ctxp_o", [BPC, 4, D], F32, kind="ExternalOutput")
    zs_o = nc.dram_tensor("zs_o", [BPC, P, 2], F32, kind="ExternalOutput")

    with tile.TileContext(nc) as tc:
        with (
            tc.tile_pool(name="const", bufs=1) as cpool,
            tc.tile_pool(name="encT", bufs=2 * NDC) as encT_pool,
            tc.tile_pool(name="encN", bufs=2 * NST) as encN_pool,
            tc.tile_pool(name="energy", bufs=8) as en_pool,
            tc.tile_pool(name="small", bufs=2) as sm_pool,
            tc.tile_pool(name="pproj", bufs=2, space="PSUM") as pp_pool,
            tc.tile_pool(name="pscore", bufs=2, space="PSUM") as ps_pool,
            tc.tile_pool(name="pmisc", bufs=1, space="PSUM") as pt_pool,
            tc.tile_pool(name="pctx", bufs=1, space="PSUM") as pc_pool,
        ):
            # constants
            we_sb = cpool.tile([P, NDC * A], BF16, tag="we")
            nc.sync.dma_start(
                out=we_sb[:].rearrange("p (dc a) -> p dc a", dc=NDC),
                in_=we_d[:].rearrange("(dc p) a -> p dc a", p=P),
            )
            v_sb = cpool.tile([P, NAC], BF16, tag="v")
            nc.sync.dma_start(out=v_sb[:], in_=v_d[:])
            bias_sb = cpool.tile([P, NAC * BPC], F32, tag="bias")
            nc.sync.dma_start(out=bias_sb[:], in_=bias_d[:])
            id_sb = cpool.tile([P, P], BF16, tag="ident")
            nc.sync.dma_start(out=id_sb[:], in_=id_d[:])

            for b in range(BPC):
                # ---- load this batch's encoder states ----
                encT_sb = []
                for dc in range(NDC):
                    t = encT_pool.tile([P, S], BF16, tag="encT")
                    nc.sync.dma_start(out=t[:], in_=encT_d[b, dc * P:(dc + 1) * P, :])
                    encT_sb.append(t)
                encN_sb = []
                for g in range(NST):
                    t = encN_pool.tile([P, 4 * D], BF16, tag="encN")
                    nc.sync.dma_start(
                        out=t[:].rearrange("p (j d) -> p j d", j=4),
                        in_=encN_d[b, g * 512:(g + 1) * 512, :].rearrange(
                            "(j p) d -> p j d", p=P),
                    )
                    encN_sb.append(t)

                # ---- projT -> tanh -> scores ----
                # scores psum: two banks, s-tile st lives at partition 32*(st%4)
                # of bank st//4.
                sc_ps = [ps_pool.tile([P, 512], F32, tag="scores",
                                      name=f"sc_ps_{b}_{g}")
                         for g in range(2)]
                for t2 in range(NST // 2):       # s-tile pairs
                    ets = []
                    for ac in range(NAC):
                        pp = pp_pool.tile([P, 1024], F32, tag="proj",
                                          name=f"pp_{b}_{t2}_{ac}")
                        for half in range(2):
                            st = 2 * t2 + half
                            for dc in range(NDC):
                                nc.tensor.matmul(
                                    pp[:, half * 512:(half + 1) * 512],
                                    lhsT=we_sb[:, dc * A + ac * P:
                                               dc * A + (ac + 1) * P],
                                    rhs=encT_sb[dc][:, st * 512:(st + 1) * 512],
                                    start=(dc == 0), stop=(dc == NDC - 1),
                                )
                        et = en_pool.tile([P, 1024], BF16, tag="energy",
                                          name=f"et_{b}_{t2}_{ac}")
                        nc.scalar.activation(
                            et[:], pp[:], AF.Tanh,
                            bias=bias_sb[:, ac * BPC + b:ac * BPC + b + 1],
                        )
                        ets.append(et)
                    # score accumulation groups must be contiguous per PSUM
                    # bank, so run each s-tile's 4 a-chunk matmuls together.
                    for half in range(2):
                        st = 2 * t2 + half
                        g, j = st // 4, st % 4
                        for ac in range(NAC):
                            nc.tensor.matmul(
                                sc_ps[g][32 * j:32 * j + 32, :],
                                lhsT=v_sb[:, ac:ac + 1].to_broadcast((P, 32)),
                                rhs=ets[ac][:, half * 512:(half + 1) * 512],
                                start=(ac == 0), stop=(ac == NAC - 1),
                                tile_position=(0, 32 * j),
                            )

                # ---- scores out + exp + softmax partials ----
                # Every psum row is a (replicated) valid score row, so exp
                # runs on full banks; accum_out gives per-row sums whose rows
                # {0,32,64,96} are the softmax Z partials (summed on host).
                exps_sb = sm_pool.tile([P, 1024], BF16, tag="exps")
                zacc_sb = sm_pool.tile([P, 2], F32, tag="zacc")
                for g in range(2):
                    sc_sb = sm_pool.tile([P, 512], F32, tag="scores_sb",
                                         name=f"sc_sb_{b}_{g}")
                    nc.vector.tensor_copy(sc_sb[:], sc_ps[g][:])
                    nc.sync.dma_start(
                        out=scores_o[b, g * 2048:(g + 1) * 2048].rearrange(
                            "(j f) -> j f", f=512),
                        in_=sc_sb[:].rearrange("(j r) f -> j (r f)", r=32)[:, 0:512],
                    )
                    nc.scalar.activation(
                        exps_sb[:, g * 512:(g + 1) * 512],
                        sc_ps[g][:],
                        AF.Exp,
                        accum_out=zacc_sb[:, g:g + 1],
                    )
                nc.sync.dma_start(out=zs_o[b], in_=zacc_sb[:])

                # ---- transpose exps to [s-partition, chunk] layout ----
                # probsT_sb[:, (g*4 + jj)*4 + j] holds s-chunk (g*4+j)*4 + jj.
                probsT_sb = sm_pool.tile([P, 32], BF16, tag="probsT_sb")
                for g in range(2):
                    for jj in range(4):
                        pt_ps = pt_pool.tile([P, P], BF16, tag="probsT",
                                             name=f"pt_{b}_{g}_{jj}")
                        nc.tensor.transpose(
                            pt_ps[:],
                            exps_sb[:, g * 512 + jj * P:g * 512 + (jj + 1) * P],
                            id_sb[:],
                        )
                        nc.vector.tensor_copy(
                            probsT_sb[:, (g * 4 + jj) * 4:(g * 4 + jj) * 4 + 4]
                            .rearrange("p (j one) -> p j one", one=1),
                            pt_ps[:].rearrange("p (j r) -> p j r", r=32)[:, :, 0:1],
                        )

                # ---- context partials (4 col-tiled accumulation groups) ----
                ctx_ps = pc_pool.tile([P, 512], F32, tag="ctx")
                for j in range(4):
                    for i in range(8):
                        c = j * 8 + i            # s chunk index
                        col = (c // 16) * 16 + (c % 4) * 4 + (c % 16) // 4
                        nc.tensor.matmul(
                            ctx_ps[32 * j:32 * j + 32, :],
                            lhsT=probsT_sb[:, col:col + 1].to_broadcast((P, 32)),
                            rhs=encN_sb[c // 4][:, (c % 4) * 512:(c % 4 + 1) * 512],
                            start=(i == 0), stop=(i == 7),
                            tile_position=(0, 32 * j),
                        )
                ctx_sb = sm_pool.tile([P, 512], F32, tag="ctx_sb")
                nc.vector.tensor_copy(ctx_sb[:], ctx_ps[:])
                nc.sync.dma_start(
                    out=ctxp_o[b],
                    in_=ctx_sb[:].rearrange("(j r) f -> j (r f)", r=32)[:, 0:512],
                )

    nc.compile()
    return nc


_NC_CACHE = None


def _get_nc():
    global _NC_CACHE
    if _NC_CACHE is None:
        _NC_CACHE = build_nc()
    return _NC_CACHE


def make_in_maps(encode_state, decode_state, W, v):
    """Host-side shard + layout prep. Returns per-core input dicts."""
    bf16 = ml_dtypes.bfloat16
    W_e, W_d = W[:D], W[D:]
    dec_proj = decode_state.astype(np.float32) @ W_d.astype(np.float32)  # [B, A]
    enc_bf = encode_state.astype(bf16)
    we_bf = np.ascontiguousarray(W_e.astype(bf16))
    v_bf = np.ascontiguousarray(v.astype(bf16).reshape(NAC, P).T)        # [P, NAC]
    ident = np.eye(P, dtype=bf16)

    in_maps = []
    for c in range(N_CORES):
        sl = slice(c * BPC, (c + 1) * BPC)
        encN = np.ascontiguousarray(enc_bf[sl])                          # [BPC,S,D]
        encT = np.ascontiguousarray(enc_bf[sl].transpose(0, 2, 1))       # [BPC,D,S]
        # biasT[p, ac*BPC + b] = dec_proj[core b, ac*128 + p]
        biasT = np.ascontiguousarray(
            dec_proj[sl].reshape(BPC, NAC, P).transpose(2, 1, 0).reshape(P, NAC * BPC)
        ).astype(np.float32)
        in_maps.append({
            "encT": encT, "encN": encN, "we": we_bf, "vv": v_bf,
            "biasT": biasT, "ident": ident,
        })
    return in_maps


def assemble(results):
    """Gather per-core outputs into full (scores, context_vector)."""
    scores = np.concatenate([r["scores_o"] for r in results], axis=0)
    ctx = []
    for r in results:
        z = r["zs_o"][:, 0:97:32, :].sum(axis=(1, 2))                    # [BPC]
        c = r["ctxp_o"].sum(axis=1) / z[:, None]                         # [BPC, D]
        ctx.append(c)
    context = np.concatenate(ctx, axis=0).astype(np.float32)
    return scores.astype(np.float32), context


def kernel(encode_state, decode_state, W, v):
    from concourse.bass_utils import run_bass_kernel_spmd
    nc = _get_nc()
    in_maps = make_in_maps(encode_state, decode_state, W, v)
    res = run_bass_kernel_spmd(nc, in_maps, list(range(N_CORES)))
    return assemble(res.results)
